# revision 30
# baseline (speedup 1.0000x reference)
"""MoE layer (E=8, top-2, SwiGLU experts) on 8 trn2 NeuronCores.

Strategy (expert-parallel with hidden-dim load balancing, host-routed):
  - Router (flat @ router_w.T, top-2, softmax) is computed on host in fp32;
    it is tiny (33 MFLOP) and must match the reference's expert selection
    exactly (min top2-vs-3rd logit gap on these inputs is ~1e-4, far above
    fp32 matmul noise ~1e-6).
  - Load balance: per-expert token counts vary (1071 max vs 1024 mean for
    the graded inputs). A pure expert-per-core layout pads every core to the
    max count. Instead each expert's FFN is split along the HIDDEN dim into
    4 quarter-jobs (4 h-slabs of 128 each); SwiGLU is elementwise in h, and
    stage 2 (y = w2 @ m) is linear in h, so quarter outputs simply ADD.
    Every core runs 4 job slots; slot k across all 8 cores holds the 8
    quarter-jobs of the two experts ranked (2k, 2k+1) by token count, so
    slot k's token capacity is the rank-2k count, not the global max:
    per-core work = sum(cap_k) * 4 slabs instead of max_count * 16.
  - Each quarter-job: dense bf16 SwiGLU over its expert's tokens,
    yq = w2[:, q].T @ (silu(w1[q].T@xT) * (w3[q].T@xT)), fp32 PSUM,
    partial outputs written in bf16 and summed (4 quarters) on host with
    the fp32 combine weights. All tensors pre-transposed AND pre-packed on
    host into SBUF-resident layouts so every device DMA is a linear copy.
  - All input and output DMAs ride the SP HWDGE ring in consumption order.
    (The ACT ring is a trap: its dma_start descriptors sit behind the Silu
    ACTIVATEs in the Scalar engine's strict-FIFO queue, delaying output
    completion by 10+us. Input loads finish by ~46us and the first output
    issues later, so there is no SP queueing conflict.)
  - HAM warmup: the PE clock gate releases to 2.4 GHz only after a full
    ~3.4us window of sustained PE activity. Junk N=512 matmuls (~430ns each
    at the cold 1.2 GHz) bridge from the framework preamble (~7.5us) to the
    first real matmul (~11us, gated on the x+w1 DMA prefix), so HAM fires
    during the bridge instead of ~8us into real work. Slot 0's first chunk
    is 384 tokens: small enough for a short x prefix, large enough that its
    stage-1 weight-consumption pace stays below the degraded early DMA
    delivery rate (~230GB/s) so the PE never stalls pre-warm (a stall there
    re-throttles HAM back to 1.2 GHz for ~3.4us).
"""

import os
import numpy as np
import ml_dtypes

B, S, D, H, E = 2, 2048, 1024, 2048, 8
T = B * S
TOP_K = 2
P = 128
NTOK = 512     # max token chunk (matmul free dim / one PSUM bank of fp32)
D_T = D // P   # 8 contraction slabs
H_T = H // P   # 16 hidden slabs per expert
SLOTS = 4      # quarter-jobs per expert
SLAB = H_T // SLOTS  # 4 h-slabs per job
SLABW = SLAB * P     # 512 hidden units per job
JUNK_MM = 12   # HAM-warmup junk matmuls (N=512, ~430ns each cold)
# slot-0 w1/w3 stream in hidden-column packs (h-slab start, count) so the
# first gate group's critical DMA prefix is one slab, not the whole job
PACKS0 = [(0, 1), (1, 1), (2, 2)]

_cache = {}

# set by the last kernel() call when tracing is enabled (KERNEL_TRACE=1)
LAST_RESULTS = None


def _chunk_plan(cap, first_small):
    """Chunk sizes for one job slot. first_small (slot 0 only) keeps the
    first chunk at 384 so the startup x prefix is small; the last chunk is
    small so the end-of-kernel tail is short. All sizes multiples of 8."""
    first = 384 if first_small else NTOK
    first = min(first, cap)
    sizes = [first]
    rem = cap - first
    if first_small:
        # slot 0: its tail is not the kernel tail; plain fill, fewest chunks
        while rem > NTOK:
            sizes.append(NTOK)
            rem -= NTOK
        if rem:
            sizes.append(rem)
    else:
        while rem > NTOK + 128:
            sizes.append(NTOK)
            rem -= NTOK
        if rem == NTOK:
            sizes.append(NTOK)      # e.g. 1024 -> [512, 512]: fewest chunks
        elif rem > 256:
            # e.g. 1040 -> [512, 400, 128]: split the rest for a short tail
            sizes.append(rem - 128)
            sizes.append(128)
        elif rem:
            sizes.append(rem)
    chunks, s = [], 0
    for n in sizes:
        chunks.append((s, n))
        s += n
    return chunks


def _pack_x(xTe, chunks):
    """[D, cap] -> [128, D_T*cap], chunk-blocked, partition-major."""
    arr = xTe.reshape(D_T, P, -1).transpose(1, 0, 2)  # [128, D_T, cap]
    blocks = [arr[:, :, s0:s0 + n].reshape(P, D_T * n) for s0, n in chunks]
    return np.ascontiguousarray(np.concatenate(blocks, axis=1))


def _pack_w13(wT, packs):
    """[D, SLABW] -> [128, D_T*SLABW], pack-ordered, pack-contiguous."""
    arr = wT.reshape(D_T, P, SLABW).transpose(1, 0, 2)  # [128, D_T, SLABW]
    blocks = [
        arr[:, :, h0 * P:(h0 + hc) * P].reshape(P, D_T * hc * P)
        for h0, hc in packs
    ]
    return np.ascontiguousarray(np.concatenate(blocks, axis=1))


def _pack_w2(w2T):
    """[SLABW, D] -> [128, SLAB*D] (h-slab-major, partition-first)."""
    arr = w2T.reshape(SLAB, P, D).transpose(1, 0, 2)    # [128, SLAB, D]
    return np.ascontiguousarray(arr.reshape(P, SLAB * D))


def _build_nc(caps, act="silu"):
    import concourse.mybir as mybir
    import concourse.tile as tile
    from concourse import bacc

    bf16 = mybir.dt.bfloat16
    f32 = mybir.dt.float32
    # "sigmoid" exists only for CoreSim smoke tests (sim lacks Silu)
    Silu = (
        mybir.ActivationFunctionType.Silu
        if act == "silu"
        else mybir.ActivationFunctionType.Sigmoid
    )

    plans = [_chunk_plan(c, k == 0) for k, c in enumerate(caps)]

    nc = bacc.Bacc()
    xT_d, w1_d, w3_d, w2_d, yT_d = [], [], [], [], []
    for k, c in enumerate(caps):
        xT_d.append(nc.declare_dram_parameter(f"xT{k}", [P, D_T * c], bf16, isOutput=False))
        w1_d.append(nc.declare_dram_parameter(f"w1T{k}", [P, D_T * SLABW], bf16, isOutput=False))
        w3_d.append(nc.declare_dram_parameter(f"w3T{k}", [P, D_T * SLABW], bf16, isOutput=False))
        w2_d.append(nc.declare_dram_parameter(f"w2T{k}", [P, SLAB * D], bf16, isOutput=False))
        yT_d.append(nc.declare_dram_parameter(f"yT{k}", [P, D_T, c], bf16, isOutput=True))
    # the very last chunk writes its own CONTIGUOUS tensor: a column slice
    # of yT gives 2*n-byte descriptors (n=128 -> 256B -> ~50GB/s); a packed
    # [P, D_T, n] tail tensor gives 2KB descriptors and drains ~8x faster,
    # which is on the critical path after the final matmul
    tail_n = plans[-1][-1][1]
    yTtail_d = nc.declare_dram_parameter("yTtail", [P, D_T, tail_n], bf16, isOutput=True)

    with tile.TileContext(nc) as tc:
        with (
            tc.tile_pool(name="w0pool", bufs=1) as w0pool,
            tc.tile_pool(name="wpool", bufs=2) as wpool,
            tc.tile_pool(name="xpool", bufs=2) as xpool,
            tc.tile_pool(name="hpool", bufs=2) as hpool,
            tc.tile_pool(name="gpool", bufs=4) as gpool,
            tc.tile_pool(name="opool", bufs=4) as opool,
            tc.tile_pool(name="pspool", bufs=2, space="PSUM") as pspool,
        ):
            # --- DMA issue, SP ring, in consumption order ---------------
            # slot 0: x chunk-0 in 2-slab slices interleaved with the first
            # w1/w3 packs (small critical prefix), then w2, then x chunks.
            n00 = plans[0][0][1]
            xs00 = xpool.tile([P, D_T, n00], bf16, tag="x", name="x00", bufs=6)
            w1s0, w3s0 = {}, {}
            w0tiles = {}
            off = 0
            for h0, hc in PACKS0:
                w = D_T * hc * P
                t1 = w0pool.tile([P, D_T, hc * P], bf16, tag=f"w1s0_{h0}", name=f"w1s0_{h0}")
                t3 = w0pool.tile([P, D_T, hc * P], bf16, tag=f"w3s0_{h0}", name=f"w3s0_{h0}")
                w0tiles[h0] = (t1, t3, off, w)
                for j in range(hc):
                    w1s0[h0 + j] = (t1, j * P)
                    w3s0[h0 + j] = (t3, j * P)
                off += w
            # critical-prefix order: chunk-0 x slices complete right after
            # the first w1/w3 packs so the first gate group never waits on
            # an x slice queued behind later weight packs
            t1, t3, o0, wl = w0tiles[0]
            nc.sync.dma_start(xs00[:, 0:2, :], xT_d[0][:, 0:2 * n00])
            nc.sync.dma_start(t1[:].rearrange("p d c -> p (d c)"), w1_d[0][:, o0:o0 + wl])
            nc.sync.dma_start(xs00[:, 2:4, :], xT_d[0][:, 2 * n00:4 * n00])
            nc.sync.dma_start(t3[:].rearrange("p d c -> p (d c)"), w3_d[0][:, o0:o0 + wl])
            nc.sync.dma_start(xs00[:, 4:6, :], xT_d[0][:, 4 * n00:6 * n00])
            nc.sync.dma_start(xs00[:, 6:8, :], xT_d[0][:, 6 * n00:8 * n00])
            for h0, hc in PACKS0[1:]:
                t1, t3, o0, wl = w0tiles[h0]
                nc.sync.dma_start(t1[:].rearrange("p d c -> p (d c)"), w1_d[0][:, o0:o0 + wl])
                nc.sync.dma_start(t3[:].rearrange("p d c -> p (d c)"), w3_d[0][:, o0:o0 + wl])
            w2t0 = w0pool.tile([P, SLAB, D], bf16, tag="w2s0", name="w2s0")
            nc.sync.dma_start(w2t0[:].rearrange("p d c -> p (d c)"), w2_d[0][:, :])

            # HAM warmup junk matmuls (see module docstring). Shares the
            # "py" psum slots (stage 2 only, first needed ~25us in) so all
            # 8 PSUM banks go to real tiles and junk never blocks stage 1.
            warm_sb = gpool.tile([P, NTOK], bf16, tag="warm_sb", name="warm_sb")
            nc.vector.memset(warm_sb[:], 0.0)
            warm_ps = pspool.tile([P, NTOK], f32, tag="py", name="warm_ps", bufs=3)
            for _ in range(JUNK_MM):
                nc.tensor.matmul(
                    warm_ps[:], lhsT=warm_sb[:, :P], rhs=warm_sb[:],
                    start=True, stop=True,
                )

            # remaining slot-0 x chunks (x tiles get 6 buffers so no issued
            # DMA ever write-after-read-waits on a buffer: a blocked DMA
            # head-of-line-blocks the in-order SP queue and with it every
            # output DMA behind it -> PSUM backpressure -> PE stalls)
            xs_tiles = {(0, 0): xs00}
            for ci in range(1, len(plans[0])):
                s0, n = plans[0][ci]
                xt = xpool.tile([P, D_T, n], bf16, tag="x", name=f"x0{ci}", bufs=6)
                nc.sync.dma_start(xt[:], xT_d[0][:, D_T * s0:D_T * (s0 + n)])
                xs_tiles[(0, ci)] = xt
            w1t = {0: None}
            w3t = {0: None}
            w2t = {0: w2t0}

            def issue_slot_loads(k):
                """Issue slot k's x + weight loads (consumption order)."""
                s0, n = plans[k][0]
                xt = xpool.tile([P, D_T, n], bf16, tag="x", name=f"x{k}0", bufs=6)
                nc.sync.dma_start(xt[:], xT_d[k][:, 0:D_T * n])
                xs_tiles[(k, 0)] = xt
                t1 = wpool.tile([P, D_T, SLABW], bf16, tag="w1s", name=f"w1s{k}")
                nc.sync.dma_start(t1[:].rearrange("p d c -> p (d c)"), w1_d[k][:, :])
                t3 = wpool.tile([P, D_T, SLABW], bf16, tag="w3s", name=f"w3s{k}")
                nc.sync.dma_start(t3[:].rearrange("p d c -> p (d c)"), w3_d[k][:, :])
                t2 = wpool.tile([P, SLAB, D], bf16, tag="w2s", name=f"w2s{k}")
                nc.sync.dma_start(t2[:].rearrange("p d c -> p (d c)"), w2_d[k][:, :])
                w1t[k] = t1
                w3t[k] = t3
                w2t[k] = t2
                for ci in range(1, len(plans[k])):
                    s0, n = plans[k][ci]
                    xt = xpool.tile([P, D_T, n], bf16, tag="x", name=f"x{k}{ci}", bufs=6)
                    nc.sync.dma_start(xt[:], xT_d[k][:, D_T * s0:D_T * (s0 + n)])
                    xs_tiles[(k, ci)] = xt

            # slot 1's loads go up front (its buffers are all fresh); slot
            # k+1's are issued as slot k's compute begins, by which time
            # slot k-1 (whose buffers slot k+1 reuses) is fully consumed.
            issue_slot_loads(1)

            # --- compute ------------------------------------------------
            for k in range(SLOTS):
                if 2 <= k + 1 < SLOTS:
                    issue_slot_loads(k + 1)
                for ci, (s0, n) in enumerate(plans[k]):
                    last_chunk = (k == SLOTS - 1 and ci == len(plans[k]) - 1)
                    xs = xs_tiles[(k, ci)]
                    # stage 1: ht[h] = silu(w1.T@xT) * (w3.T@xT), [128,n] bf16
                    hts = []
                    for h in range(SLAB):
                        if k == 0:
                            t1, c1 = w1s0[h]
                            t3, c3 = w3s0[h]
                        else:
                            t1, c1 = w1t[k], h * P
                            t3, c3 = w3t[k], h * P
                        pg = pspool.tile([P, NTOK], f32, tag="pg", name="pg")
                        for d in range(D_T):
                            nc.tensor.matmul(
                                pg[:, :n],
                                lhsT=t1[:, d, c1:c1 + P],
                                rhs=xs[:, d, :],
                                start=(d == 0),
                                stop=(d == D_T - 1),
                            )
                        pu = pspool.tile([P, NTOK], f32, tag="pu", name="pu", bufs=3)
                        for d in range(D_T):
                            nc.tensor.matmul(
                                pu[:, :n],
                                lhsT=t3[:, d, c3:c3 + P],
                                rhs=xs[:, d, :],
                                start=(d == 0),
                                stop=(d == D_T - 1),
                            )
                        g = gpool.tile([P, NTOK], bf16, tag="g", name="g")
                        nc.scalar.activation(g[:, :n], pg[:, :n], Silu)
                        ht = hpool.tile([P, NTOK], bf16, tag=f"h_{h}", name=f"h_{h}")
                        nc.vector.tensor_mul(out=ht[:, :n], in0=g[:, :n], in1=pu[:, :n])
                        hts.append(ht)

                    # stage 2: yq[do] = sum_h w2[h,do].T @ ht[h] -> [128,n].
                    # The 8 do-outputs accumulate in ONE [P, D_T, n] tile and
                    # leave in ONE chunk-wide DMA: each dma_start costs
                    # ~650ns of SP sequencer descriptor-gen, and 8 per chunk
                    # outpaces the PE on small chunks (head-of-line blocks
                    # the in-order SP queue -> PSUM backpressure).
                    # two half-DMAs per chunk: the do 0-3 half leaves while
                    # do 4-7 still computes, halving the after-last-matmul
                    # output drain at the end of the kernel
                    ot = opool.tile([P, D_T, NTOK], bf16, tag="o", name="o", bufs=2)
                    for do in range(D_T):
                        py = pspool.tile([P, NTOK], f32, tag="py", name="py", bufs=3)
                        for h in range(SLAB):
                            nc.tensor.matmul(
                                py[:, :n],
                                lhsT=w2t[k][:, h, do * P:(do + 1) * P],
                                rhs=hts[h][:, :n],
                                start=(h == 0),
                                stop=(h == SLAB - 1),
                            )
                        nc.vector.tensor_copy(ot[:, do, :n], py[:, :n])
                        if last_chunk:
                            # quarter DMAs: all but the last 64KB leave while
                            # the remaining matmuls still run, minimizing the
                            # post-final-matmul drain
                            if do % 2 == 1:
                                nc.sync.dma_start(
                                    yTtail_d[:, do - 1:do + 1, :n],
                                    ot[:, do - 1:do + 1, :n],
                                )
                        elif do == D_T // 2 - 1:
                            nc.sync.dma_start(
                                yT_d[k][:, 0:D_T // 2, s0:s0 + n],
                                ot[:, 0:D_T // 2, :n],
                            )
                    if not last_chunk:
                        nc.sync.dma_start(
                            yT_d[k][:, D_T // 2:, s0:s0 + n], ot[:, D_T // 2:, :n]
                        )

    nc.finalize()
    return nc


def kernel(x, router_w, w1, w2, w3):
    global LAST_RESULTS
    from concourse.bass_utils import run_bass_kernel_spmd

    x = np.ascontiguousarray(np.asarray(x, dtype=np.float32))
    router_w = np.asarray(router_w, dtype=np.float32)
    flat = x.reshape(T, D)

    # ---- host router (fp32, matches reference math) ----
    logits = flat @ router_w.T                      # [T, E]
    rows = np.arange(T)
    i1 = np.argmax(logits, axis=1)
    v1 = logits[rows, i1]
    masked = logits.copy()
    masked[rows, i1] = -np.inf
    i2 = np.argmax(masked, axis=1)
    v2 = masked[rows, i2]
    # softmax over the two selected logits (v1 >= v2)
    e2 = np.exp(v2 - v1)
    wt1 = 1.0 / (1.0 + e2)
    wt2 = e2 / (1.0 + e2)

    # ---- dispatch: token lists per expert ----
    idxs, wts = [], []
    for e in range(E):
        m1 = i1 == e
        m2 = i2 == e
        idx = np.nonzero(m1 | m2)[0]
        w = np.where(m1[idx], wt1[idx], wt2[idx]).astype(np.float32)
        idxs.append(idx)
        wts.append(w)
    cnt = np.array([len(i) for i in idxs])

    # slot k holds the two experts ranked (2k, 2k+1) by count; its token
    # capacity is the larger of the pair, rounded up to a multiple of 8
    order = np.argsort(-cnt, kind="stable")         # expert ids, desc count
    pairs = [(int(order[2 * k]), int(order[2 * k + 1])) for k in range(SLOTS)]
    caps = tuple(max(NTOK, -(-int(cnt[p[0]]) // 8) * 8) for p in pairs)
    plans = [_chunk_plan(c, k == 0) for k, c in enumerate(caps)]

    if caps not in _cache:
        _cache[caps] = _build_nc(caps)
    nc = _cache[caps]

    # ---- per-core inputs (bf16, pre-transposed, pre-packed) ----
    bf = ml_dtypes.bfloat16
    xpacks = {}                                     # expert -> packed x
    for k, (ea, eb) in enumerate(pairs):
        for e in (ea, eb):
            idx = idxs[e]
            xTe = np.zeros((D, caps[k]), dtype=bf)
            xTe[:, :len(idx)] = flat[idx].T.astype(bf)
            xpacks[e] = _pack_x(xTe, plans[k])

    in_maps = []
    core_expert = []                                # core -> [expert per slot]
    for c in range(E):
        m = {}
        ces = []
        q = c % 4                                   # hidden quarter
        for k, (ea, eb) in enumerate(pairs):
            e = ea if c < 4 else eb
            ces.append(e)
            w1b = np.ascontiguousarray(w1[e].T[:, q * SLABW:(q + 1) * SLABW]).astype(bf)
            w3b = np.ascontiguousarray(w3[e].T[:, q * SLABW:(q + 1) * SLABW]).astype(bf)
            w2b = np.ascontiguousarray(w2[e].T[q * SLABW:(q + 1) * SLABW, :]).astype(bf)
            packs = PACKS0 if k == 0 else [(0, SLAB)]
            m[f"xT{k}"] = xpacks[e]
            m[f"w1T{k}"] = _pack_w13(w1b, packs)
            m[f"w3T{k}"] = _pack_w13(w3b, packs)
            m[f"w2T{k}"] = _pack_w2(w2b)
        in_maps.append(m)
        core_expert.append(ces)

    trace = os.environ.get("KERNEL_TRACE", "0") == "1"
    kwargs = {}
    if trace:
        kwargs = dict(trace=True, trace_cores=list(range(E)))
    res = run_bass_kernel_spmd(nc, in_maps, core_ids=list(range(E)), **kwargs)
    LAST_RESULTS = res

    # ---- combine: sum the 4 hidden-quarter partials, apply weights ----
    out = np.zeros((T, D), dtype=np.float32)
    for k, (ea, eb) in enumerate(pairs):
        for e, cores in ((ea, range(0, 4)), (eb, range(4, 8))):
            idx = idxs[e]
            acc = np.zeros((len(idx), D), dtype=np.float32)
            for c in cores:
                yT = res.results[c][f"yT{k}"]       # [P, D_T, cap_k] bf16
                if k == SLOTS - 1:
                    ts0, tn = plans[k][-1]          # last chunk -> yTtail
                    yT = np.concatenate(
                        [yT[:, :, :ts0], res.results[c]["yTtail"]], axis=2
                    )
                yD = yT.transpose(1, 0, 2).reshape(D, caps[k])
                acc += yD[:, :len(idx)].T.astype(np.float32)
            out[idx] += wts[e][:, None] * acc
    return out.reshape(B, S, D)
